# revision 40
# baseline (speedup 1.0000x reference)
"""TRN2 Bass kernel for CustomScaledDotProductAttention.

Sharding: 8 cores = 4 batches x 2 head-groups (tensor-parallel over heads).
Each core computes, for its (batch b, head-group g of 8 heads):
  - Q^T/K^T projections in d-major layout [o=512, n=1024] (fp32r matmuls),
    with 1/sqrt(dh) folded into Wq host-side,
  - V projection in natural layout [n, o] with bias via an fp32 ones-matmul,
    augmented with a ones-column per head (softmax denominators fall out of
    the attention*V matmul for free),
  - q-major scores -> additive-mask (bf16 0/-30) -> Exp with accumulated row
    sums -> reciprocal -> normalize -> distribution output,
  - k-major scores -> Exp -> multiplicative mask (bf16 0/1, gpsimd) ->
    attention*V (fp32r) giving ctx^T and row sums,
  - ctx normalization via a PE ones-broadcast of the reciprocal sum row,
  - output projection (fp32r) producing this group's partial results.
Host glue: transposes/rounds inputs, sums the two partial results per batch,
adds bo, and scatters per-core distribution slices into the full output.
"""
import os
import sys

for _p in ("/opt/trn_rl_repo",):
    if os.path.isdir(_p) and _p not in sys.path:
        sys.path.insert(0, _p)

import numpy as np
import ml_dtypes
from contextlib import ExitStack

import concourse.bass as bass
import concourse.tile as tile
from concourse import bacc, mybir
from concourse.bass_utils import run_bass_kernel_spmd

B, N, F, HID, H, DH = 4, 1024, 1024, 1024, 16, 64
NCORES, G = 8, 2
HG = H // G          # heads per group
S = HID // G         # hidden slice per group
P = 128
KC = F // P          # contraction chunks
NEG = -30.0          # additive mask value (exp(-30+s) ~ 1e-13, negligible)

F32 = mybir.dt.float32
F32R = mybir.dt.float32r
BF16 = mybir.dt.bfloat16
F8 = mybir.dt.float8e4
EXP = mybir.ActivationFunctionType.Exp
IDENT = mybir.ActivationFunctionType.Identity
MULT = mybir.AluOpType.mult
ADD = mybir.AluOpType.add

_CACHE = {}


def _round_fp32r(x):
    u = np.ascontiguousarray(x, dtype=np.float32).view(np.uint32)
    r = ((u + 0x800 + ((u >> 12) & 1)) & 0xFFFFF000).astype(np.uint32)
    return r.view(np.float32)


def build_program():
    nc = bacc.Bacc("TRN2", target_bir_lowering=False, debug=False,
                   num_devices=NCORES)

    xqT = nc.dram_tensor("xqT", [F, N], F32R, kind="ExternalInput").ap()
    xkT = nc.dram_tensor("xkT", [F, N], F32R, kind="ExternalInput").ap()
    xvT = nc.dram_tensor("xvT", [F, N], F32R, kind="ExternalInput").ap()
    wqT = nc.dram_tensor("wqT", [F, S], F32R, kind="ExternalInput").ap()
    wkT = nc.dram_tensor("wkT", [F, S], F32R, kind="ExternalInput").ap()
    wvT = nc.dram_tensor("wvT", [F, S], F32R, kind="ExternalInput").ap()
    bqr = nc.dram_tensor("bqr", [P, S // P], F32, kind="ExternalInput").ap()
    bkr = nc.dram_tensor("bkr", [P, S // P], F32, kind="ExternalInput").ap()
    bvr = nc.dram_tensor("bvr", [1, S], F32, kind="ExternalInput").ap()
    woT = nc.dram_tensor("woT", [S, HID], F32R, kind="ExternalInput").ap()
    madd = nc.dram_tensor("madd", [N, N], BF16, kind="ExternalInput").ap()
    mmulT = nc.dram_tensor("mmulT", [N, N], BF16, kind="ExternalInput").ap()
    dist = nc.dram_tensor("dist", [N, HG, N], F32, kind="ExternalOutput").ap()
    res = nc.dram_tensor("res", [N, HID], F32, kind="ExternalOutput").ap()

    with tile.TileContext(nc) as tc, ExitStack() as ctx:
        resA = ctx.enter_context(tc.tile_pool(name="resA", bufs=1))
        WO = resA.tile([P, S // P, HID], F32R, tag="WO")
        CTX = resA.tile([P, S // P, N], F32R, tag="CTX")
        ONES = resA.tile([1, P], F32, tag="ONES")
        ONES64 = resA.tile([P, HG * KC], F32, tag="ONES64")
        BQ = resA.tile([P, S // P], F32, tag="BQ")
        BK = resA.tile([P, S // P], F32, tag="BK")
        BV = resA.tile([1, S], F32, tag="BV")

        nc.sync.dma_start(BQ[:], bqr)
        nc.sync.dma_start(BK[:], bkr)
        nc.sync.dma_start(BV[:], bvr)
        nc.vector.memset(ONES[:], 1.0)
        nc.vector.memset(ONES64[:], 1.0)

        with ExitStack() as ctxB:
            resB = ctxB.enter_context(tc.tile_pool(name="resB", bufs=1))
            QT = resB.tile([P, S // P, N], F32R, tag="QT")
            KT = resB.tile([P, S // P, N], F32R, tag="KT")
            V = resB.tile([P, KC, HG, DH + 1], F32R, tag="V")
            MADD = resB.tile([P, N // P, N], BF16, tag="MADD")
            MMUL = resB.tile([P, N // P, N], BF16, tag="MMUL")
            # ones column of V_aug: ACT copy f32 -> f32r, one strided op
            nc.scalar.copy(V[:, :, :, DH:DH + 1], ONES64[:])

            # ---------------- projections ----------------
            with ExitStack() as ctxP, \
                 tc.tile_pool(name="win", bufs=2) as wpool, \
                 tc.tile_pool(name="xin", bufs=6) as xpool, \
                 tc.tile_pool(name="pproj", bufs=8, space="PSUM") as ppj:
                del ctxP
                # Q and K: d-major out, OUT[o, n] = sum_f W[f, o] X[f, n]
                for xdram, wdram, brt, OUT in ((xqT, wqT, BQ, QT),
                                               (xkT, wkT, BK, KT)):
                    wt = wpool.tile([P, KC, S], F32R, tag="w")
                    nc.sync.dma_start(wt[:], wdram.rearrange("(c p) o -> p c o", p=P))
                    psums = [ppj.tile([P, 512], F32, tag="pp", name=f"pp{i}") for i in range(8)]
                    for fc in range(KC):
                        xt = xpool.tile([P, N], F32R, tag="x")
                        nc.sync.dma_start(xt[:], xdram[fc * P:(fc + 1) * P, :])
                        for m in range(4):
                            for nh in range(2):
                                nc.tensor.matmul(
                                    psums[m * 2 + nh][:],
                                    lhsT=wt[:, fc, m * P:(m + 1) * P],
                                    rhs=xt[:, nh * 512:(nh + 1) * 512],
                                    start=(fc == 0), stop=(fc == KC - 1))
                    for m in range(4):
                        for nh in range(2):
                            nc.scalar.activation(
                                OUT[:, m, nh * 512:(nh + 1) * 512],
                                psums[m * 2 + nh][:], IDENT,
                                bias=brt[:, m:m + 1], scale=1.0)
                # V: natural out, V[n, o] = sum_f X[f, n] W[f, o]  (+ bv)
                wt = wpool.tile([P, KC, S], F32R, tag="w")
                nc.sync.dma_start(wt[:], wvT.rearrange("(c p) o -> p c o", p=P))
                psums = [ppj.tile([P, 512], F32, tag="pp", name=f"pp{i}") for i in range(8)]
                for fc in range(KC):
                    xt = xpool.tile([P, N], F32R, tag="x")
                    nc.sync.dma_start(xt[:], xvT[fc * P:(fc + 1) * P, :])
                    for nt in range(8):
                        nc.tensor.matmul(
                            psums[nt][:],
                            lhsT=xt[:, nt * P:(nt + 1) * P],
                            rhs=wt[:, fc, :],
                            start=(fc == 0), stop=False,
                            skip_group_check=True)
                for nt in range(8):
                    # bias add: plain-fp32 K=1 matmul of ones x bv row
                    nc.tensor.matmul(psums[nt][:], lhsT=ONES[:, :],
                                     rhs=BV[:, :], start=False, stop=True,
                                     skip_group_check=True)
                    nc.vector.tensor_copy(V[:, nt, :, 0:DH], psums[nt][:])

            nc.sync.dma_start(MADD[:], madd.rearrange("(c p) k -> p c k", p=P))
            nc.sync.dma_start(MMUL[:], mmulT.rearrange("(c p) q -> p c q", p=P))
            nc.sync.dma_start(WO[:], woT.rearrange("(c p) o -> p c o", p=P))

            # ---------------- attention ----------------
            # Heads processed in pairs (j0=2p, j1=2p+1): their 64-row lhsT
            # slices sit on disjoint PE row-groups (base partitions 0/64), so
            # the score matmuls run concurrently in the array. q- and k-side
            # iterations are interleaved so DVE (mask-add, normalize), ACT
            # (both exps), Pool (k-mask) and PE all have work in flight.
            with ExitStack() as ctxA, \
                 tc.tile_pool(name="spsum", bufs=4, space="PSUM") as sppool, \
                 tc.tile_pool(name="cpsum", bufs=2, space="PSUM") as cpool, \
                 tc.tile_pool(name="eq", bufs=6) as eqpool, \
                 tc.tile_pool(name="ee", bufs=4) as eepool, \
                 tc.tile_pool(name="ek", bufs=5) as ekpool, \
                 tc.tile_pool(name="rr", bufs=2) as rrpool, \
                 tc.tile_pool(name="bcp", bufs=2) as bcpool, \
                 tc.tile_pool(name="sm", bufs=8) as smpool:
                del ctxA

                def q_front(j, qt):
                    # PE scores -> DVE mask-add -> ACT exp(+sums)
                    c, hp = j // 2, (j % 2) * DH
                    esc = eqpool.tile([P, N], F32, tag="esc", name=f"esc{j}_{qt}")
                    for kh in range(2):
                        sq = sppool.tile([P, 512], F32, tag="sp",
                                         name=f"sq{j}_{qt}_{kh}")
                        nc.tensor.matmul(
                            sq[:],
                            lhsT=QT[hp:hp + DH, c, qt * P:(qt + 1) * P],
                            rhs=KT[hp:hp + DH, c, kh * 512:(kh + 1) * 512],
                            start=True, stop=True)
                        nc.vector.tensor_tensor(
                            esc[:, kh * 512:(kh + 1) * 512], sq[:],
                            MADD[:, qt, kh * 512:(kh + 1) * 512], ADD)
                    eexp = eepool.tile([P, N], F32, tag="eexp", name=f"ee{j}_{qt}")
                    sums = smpool.tile([P, 1], F32, tag="sums", name=f"sm{j}_{qt}")
                    nc.scalar.activation(eexp[:], esc[:], EXP, accum_out=sums[:])
                    return eexp, sums

                def q_back(j, qt, st):
                    # one step later: DVE recip+normalize, DMA out (no
                    # head-of-line blocking on the in-order DVE queue)
                    eexp, sums = st
                    rec = smpool.tile([P, 1], F32, tag="rec", name=f"rc{j}_{qt}")
                    nc.vector.reciprocal(rec[:], sums[:])
                    enorm = eqpool.tile([P, N], F32, tag="esc", name=f"en{j}_{qt}")
                    nc.vector.tensor_scalar_mul(enorm[:], eexp[:], rec[:])
                    nc.sync.dma_start(dist[qt * P:(qt + 1) * P, j, :], enorm[:])

                def k_front(j, kt):
                    # PE scores^T -> ACT exp -> Pool in-place mask
                    c, hp = j // 2, (j % 2) * DH
                    ekx = ekpool.tile([P, N], F32R, tag="ekx", name=f"ek{j}_{kt}")
                    for qh in range(2):
                        sk = sppool.tile([P, 512], F32, tag="sp",
                                         name=f"sk{j}_{kt}_{qh}")
                        nc.tensor.matmul(
                            sk[:],
                            lhsT=KT[hp:hp + DH, c, kt * P:(kt + 1) * P],
                            rhs=QT[hp:hp + DH, c, qh * 512:(qh + 1) * 512],
                            start=True, stop=True)
                        nc.scalar.activation(ekx[:, qh * 512:(qh + 1) * 512],
                                             sk[:], EXP)
                    # multiplicative mask in place (gpsimd reads/writes ekx);
                    # out keeps the f32r dtype so the AV matmul's producer
                    # check passes, input is read as plain f32 bits
                    nc.gpsimd.tensor_tensor(ekx[:], ekx.bitcast(F32)[:, :],
                                            MMUL[:, kt, :], MULT)
                    return ekx

                def k_av(j, kt, ctxp, ekx):
                    for qh in range(2):
                        nc.tensor.matmul(
                            ctxp[:, qh * 512:(qh + 1) * 512],
                            lhsT=V[:, kt, j, :],
                            rhs=ekx[:, qh * 512:(qh + 1) * 512],
                            start=(kt == 0), stop=(kt == KC - 1))

                def tail(j, ctxp):
                    c, hp = j // 2, (j % 2) * DH
                    rrow = rrpool.tile([1, N], F32, tag="rrow", name=f"rr{j}")
                    nc.vector.reciprocal(rrow[:], ctxp[DH:DH + 1, :])
                    # broadcast the reciprocal sums row across partitions
                    bc = bcpool.tile([DH, N], F32, tag="bc", name=f"bc{j}")
                    nc.gpsimd.partition_broadcast(bc[:], rrow[0:1, :])
                    nc.vector.tensor_tensor(CTX[hp:hp + DH, c, :],
                                            ctxp[0:DH, :], bc[:], MULT)

                # flat software pipeline across all pairs: fronts of step s
                # are emitted alongside the backs of step s-1, with no drain
                # at pair boundaries
                qs, ek, ctxs = {}, {}, {}
                steps = [(2 * p, 2 * p + 1, t)
                         for p in range(HG // 2) for t in range(8)]
                prev = None
                for j0, j1, t in steps:
                    if t == 0:
                        ctxs[j0] = cpool.tile([DH + 1, N], F32, tag="ctx",
                                              name=f"cx{j0}")
                        ctxs[j1] = cpool.tile([DH + 1, N], F32, tag="ctx",
                                              name=f"cx{j1}")
                    ek[(j0, t)] = k_front(j0, t)
                    qs[(j0, t)] = q_front(j0, t)
                    ek[(j1, t)] = k_front(j1, t)
                    qs[(j1, t)] = q_front(j1, t)
                    if t >= 1:
                        k_av(j0, t - 1, ctxs[j0], ek.pop((j0, t - 1)))
                        k_av(j1, t - 1, ctxs[j1], ek.pop((j1, t - 1)))
                        q_back(j0, t - 1, qs.pop((j0, t - 1)))
                        q_back(j1, t - 1, qs.pop((j1, t - 1)))
                    elif prev is not None:
                        p0, p1 = prev
                        k_av(p0, 7, ctxs[p0], ek.pop((p0, 7)))
                        k_av(p1, 7, ctxs[p1], ek.pop((p1, 7)))
                        q_back(p0, 7, qs.pop((p0, 7)))
                        q_back(p1, 7, qs.pop((p1, 7)))
                        tail(p0, ctxs.pop(p0))
                        tail(p1, ctxs.pop(p1))
                    prev = (j0, j1)
                p0, p1 = prev
                k_av(p0, 7, ctxs[p0], ek.pop((p0, 7)))
                k_av(p1, 7, ctxs[p1], ek.pop((p1, 7)))
                q_back(p0, 7, qs.pop((p0, 7)))
                q_back(p1, 7, qs.pop((p1, 7)))
                tail(p0, ctxs.pop(p0))
                tail(p1, ctxs.pop(p1))

        # ---------------- output projection ----------------
        with tc.tile_pool(name="opsum", bufs=4, space="PSUM") as oppool, \
             tc.tile_pool(name="ores", bufs=16) as orpool:
            for qt in range(8):
                for oh in range(2):
                    ps = oppool.tile([P, 512], F32, tag="op")
                    for hc in range(S // P):
                        nc.tensor.matmul(
                            ps[:],
                            lhsT=CTX[:, hc, qt * P:(qt + 1) * P],
                            rhs=WO[:, hc, oh * 512:(oh + 1) * 512],
                            start=(hc == 0), stop=(hc == S // P - 1))
                    rs = orpool.tile([P, 512], F32, tag="ores")
                    nc.scalar.copy(rs[:], ps[:])
                    nc.sync.dma_start(res[qt * P:(qt + 1) * P,
                                          oh * 512:(oh + 1) * 512], rs[:])

    nc.finalize()
    return nc


def prep_core_inputs(c, queries, keys, values, mask, Wq, bq, Wk, bk, Wv, bv,
                     Wo, bo):
    b, g = divmod(c, G)
    sl = slice(g * S, (g + 1) * S)
    r = _round_fp32r
    scale = 1.0 / np.sqrt(DH).astype(np.float32)
    return {
        "xqT": r(np.asarray(queries[b]).T),
        "xkT": r(np.asarray(keys[b]).T),
        "xvT": r(np.asarray(values[b]).T),
        "wqT": r((np.asarray(Wq[sl]) * scale).T),
        "wkT": r(np.asarray(Wk[sl]).T),
        "wvT": r(np.asarray(Wv[sl]).T),
        "bqr": np.ascontiguousarray(
            (np.asarray(bq[sl]) * scale).reshape(S // P, P).T.astype(np.float32)),
        "bkr": np.ascontiguousarray(
            np.asarray(bk[sl]).reshape(S // P, P).T.astype(np.float32)),
        "bvr": np.asarray(bv[sl]).reshape(1, S).astype(np.float32),
        "woT": r(np.asarray(Wo[:, sl]).T),
        "madd": np.where(np.asarray(mask[b]), np.float32(0.0),
                         np.float32(NEG)).astype(ml_dtypes.bfloat16),
        "mmulT": np.ascontiguousarray(
            np.asarray(mask[b]).T.astype(ml_dtypes.bfloat16)),
    }


def _ensure_axon_backend():
    # The SPMD runner executes via jax's axon backend; make sure a stray
    # JAX_PLATFORMS=cpu (used for running references) doesn't hide it.
    if os.environ.get("JAX_PLATFORMS", "").strip() == "cpu":
        os.environ.pop("JAX_PLATFORMS")
    try:
        import jax
        if not any("NC_" in str(d) for d in jax.devices()):
            jax.config.update("jax_platforms", "axon,cpu")
    except Exception:
        pass


def run_cores(in_maps, **kwargs):
    _ensure_axon_backend()
    if "nc" not in _CACHE:
        _CACHE["nc"] = build_program()
    return run_bass_kernel_spmd(_CACHE["nc"], in_maps, list(range(NCORES)),
                                **kwargs)


def kernel(queries, keys, values, mask, Wq, bq, Wk, bk, Wv, bv, Wo, bo):
    args = (queries, keys, values, mask, Wq, bq, Wk, bk, Wv, bv, Wo, bo)
    in_maps = [prep_core_inputs(c, *args) for c in range(NCORES)]
    outs = run_cores(in_maps).results

    results = np.empty((B, N, HID), np.float32)
    distribution = np.empty((B, N, H, N), np.float32)
    bo_np = np.asarray(bo, dtype=np.float32)
    for c in range(NCORES):
        b, g = divmod(c, G)
        distribution[b, :, g * HG:(g + 1) * HG, :] = outs[c]["dist"]
    for b in range(B):
        results[b] = outs[2 * b]["res"] + outs[2 * b + 1]["res"] + bo_np
    return results, distribution


# revision 41
# speedup vs baseline: 1.0073x; 1.0073x over previous
"""TRN2 Bass kernel for CustomScaledDotProductAttention.

Sharding: 8 cores = 4 batches x 2 head-groups (tensor-parallel over heads).
Each core computes, for its (batch b, head-group g of 8 heads):
  - Q^T/K^T projections in d-major layout [o=512, n=1024] (fp32r matmuls),
    with 1/sqrt(dh) folded into Wq host-side,
  - V projection in natural layout [n, o] with bias via an fp32 ones-matmul,
    augmented with a ones-column per head (softmax denominators fall out of
    the attention*V matmul for free),
  - q-major scores -> additive-mask (bf16 0/-30) -> Exp with accumulated row
    sums -> reciprocal -> normalize -> distribution output,
  - k-major scores -> Exp -> multiplicative mask (bf16 0/1, gpsimd) ->
    attention*V (fp32r) giving ctx^T and row sums,
  - ctx normalization via a PE ones-broadcast of the reciprocal sum row,
  - output projection (fp32r) producing this group's partial results.
Host glue: transposes/rounds inputs, sums the two partial results per batch,
adds bo, and scatters per-core distribution slices into the full output.
"""
import os
import sys

for _p in ("/opt/trn_rl_repo",):
    if os.path.isdir(_p) and _p not in sys.path:
        sys.path.insert(0, _p)

import numpy as np
import ml_dtypes
from contextlib import ExitStack

import concourse.bass as bass
import concourse.tile as tile
from concourse import bacc, mybir
from concourse.bass_utils import run_bass_kernel_spmd

B, N, F, HID, H, DH = 4, 1024, 1024, 1024, 16, 64
NCORES, G = 8, 2
HG = H // G          # heads per group
S = HID // G         # hidden slice per group
P = 128
KC = F // P          # contraction chunks
NEG = -30.0          # additive mask value (exp(-30+s) ~ 1e-13, negligible)

F32 = mybir.dt.float32
F32R = mybir.dt.float32r
BF16 = mybir.dt.bfloat16
F8 = mybir.dt.float8e4
EXP = mybir.ActivationFunctionType.Exp
IDENT = mybir.ActivationFunctionType.Identity
MULT = mybir.AluOpType.mult
ADD = mybir.AluOpType.add

_CACHE = {}


def _round_fp32r(x):
    u = np.ascontiguousarray(x, dtype=np.float32).view(np.uint32)
    r = ((u + 0x800 + ((u >> 12) & 1)) & 0xFFFFF000).astype(np.uint32)
    return r.view(np.float32)


def build_program():
    nc = bacc.Bacc("TRN2", target_bir_lowering=False, debug=False,
                   num_devices=NCORES)

    xqT = nc.dram_tensor("xqT", [F, N], F32R, kind="ExternalInput").ap()
    xkT = nc.dram_tensor("xkT", [F, N], F32R, kind="ExternalInput").ap()
    xvT = nc.dram_tensor("xvT", [F, N], F32R, kind="ExternalInput").ap()
    wqT = nc.dram_tensor("wqT", [F, S], F32R, kind="ExternalInput").ap()
    wkT = nc.dram_tensor("wkT", [F, S], F32R, kind="ExternalInput").ap()
    wvT = nc.dram_tensor("wvT", [F, S], F32R, kind="ExternalInput").ap()
    bqr = nc.dram_tensor("bqr", [P, S // P], F32, kind="ExternalInput").ap()
    bkr = nc.dram_tensor("bkr", [P, S // P], F32, kind="ExternalInput").ap()
    bvr = nc.dram_tensor("bvr", [1, S], F32, kind="ExternalInput").ap()
    woT = nc.dram_tensor("woT", [S, HID], F32R, kind="ExternalInput").ap()
    madd = nc.dram_tensor("madd", [N, N], BF16, kind="ExternalInput").ap()
    mmulT = nc.dram_tensor("mmulT", [N, N], BF16, kind="ExternalInput").ap()
    dist = nc.dram_tensor("dist", [N, HG, N], F32, kind="ExternalOutput").ap()
    res = nc.dram_tensor("res", [N, HID], F32, kind="ExternalOutput").ap()

    with tile.TileContext(nc) as tc, ExitStack() as ctx:
        resA = ctx.enter_context(tc.tile_pool(name="resA", bufs=1))
        WO = resA.tile([P, S // P, HID], F32R, tag="WO")
        CTX = resA.tile([P, S // P, N], F32R, tag="CTX")
        ONES = resA.tile([1, P], F32, tag="ONES")
        ONES64 = resA.tile([P, HG * KC], F32, tag="ONES64")
        BQ = resA.tile([P, S // P], F32, tag="BQ")
        BK = resA.tile([P, S // P], F32, tag="BK")
        BV = resA.tile([1, S], F32, tag="BV")

        nc.vector.memset(ONES[:], 1.0)
        nc.vector.memset(ONES64[:], 1.0)

        with ExitStack() as ctxB:
            resB = ctxB.enter_context(tc.tile_pool(name="resB", bufs=1))
            QT = resB.tile([P, S // P, N], F32R, tag="QT")
            KT = resB.tile([P, S // P, N], F32R, tag="KT")
            V = resB.tile([P, KC, HG, DH + 1], F32R, tag="V")
            MADD = resB.tile([P, N // P, N], BF16, tag="MADD")
            MMUL = resB.tile([P, N // P, N], BF16, tag="MMUL")
            # ones column of V_aug: ACT copy f32 -> f32r, one strided op
            nc.scalar.copy(V[:, :, :, DH:DH + 1], ONES64[:])

            # ---------------- projections ----------------
            with ExitStack() as ctxP, \
                 tc.tile_pool(name="win", bufs=2) as wpool, \
                 tc.tile_pool(name="xin", bufs=6) as xpool, \
                 tc.tile_pool(name="pproj", bufs=8, space="PSUM") as ppj:
                del ctxP
                # Q and K: d-major out, OUT[o, n] = sum_f W[f, o] X[f, n]
                for xdram, wdram, brt, OUT in ((xqT, wqT, BQ, QT),
                                               (xkT, wkT, BK, KT)):
                    wt = wpool.tile([P, KC, S], F32R, tag="w")
                    nc.sync.dma_start(wt[:], wdram.rearrange("(c p) o -> p c o", p=P))
                    psums = [ppj.tile([P, 512], F32, tag="pp", name=f"pp{i}") for i in range(8)]
                    for fc in range(KC):
                        xt = xpool.tile([P, N], F32R, tag="x")
                        nc.sync.dma_start(xt[:], xdram[fc * P:(fc + 1) * P, :])
                        for m in range(4):
                            for nh in range(2):
                                nc.tensor.matmul(
                                    psums[m * 2 + nh][:],
                                    lhsT=wt[:, fc, m * P:(m + 1) * P],
                                    rhs=xt[:, nh * 512:(nh + 1) * 512],
                                    start=(fc == 0), stop=(fc == KC - 1))
                    if OUT is QT:
                        # tiny bias loads queue behind Q-proj's bulk DMAs;
                        # they are only consumed by the copybacks below
                        nc.sync.dma_start(BQ[:], bqr)
                        nc.sync.dma_start(BK[:], bkr)
                        nc.sync.dma_start(BV[:], bvr)
                    for m in range(4):
                        for nh in range(2):
                            nc.scalar.activation(
                                OUT[:, m, nh * 512:(nh + 1) * 512],
                                psums[m * 2 + nh][:], IDENT,
                                bias=brt[:, m:m + 1], scale=1.0)
                # V: natural out, V[n, o] = sum_f X[f, n] W[f, o]  (+ bv)
                wt = wpool.tile([P, KC, S], F32R, tag="w")
                nc.sync.dma_start(wt[:], wvT.rearrange("(c p) o -> p c o", p=P))
                psums = [ppj.tile([P, 512], F32, tag="pp", name=f"pp{i}") for i in range(8)]
                for fc in range(KC):
                    xt = xpool.tile([P, N], F32R, tag="x")
                    nc.sync.dma_start(xt[:], xvT[fc * P:(fc + 1) * P, :])
                    for nt in range(8):
                        nc.tensor.matmul(
                            psums[nt][:],
                            lhsT=xt[:, nt * P:(nt + 1) * P],
                            rhs=wt[:, fc, :],
                            start=(fc == 0), stop=False,
                            skip_group_check=True)
                for nt in range(8):
                    # bias add: plain-fp32 K=1 matmul of ones x bv row
                    nc.tensor.matmul(psums[nt][:], lhsT=ONES[:, :],
                                     rhs=BV[:, :], start=False, stop=True,
                                     skip_group_check=True)
                    nc.vector.tensor_copy(V[:, nt, :, 0:DH], psums[nt][:])

            nc.sync.dma_start(MADD[:], madd.rearrange("(c p) k -> p c k", p=P))
            nc.sync.dma_start(MMUL[:], mmulT.rearrange("(c p) q -> p c q", p=P))
            nc.sync.dma_start(WO[:], woT.rearrange("(c p) o -> p c o", p=P))

            # ---------------- attention ----------------
            # Heads processed in pairs (j0=2p, j1=2p+1): their 64-row lhsT
            # slices sit on disjoint PE row-groups (base partitions 0/64), so
            # the score matmuls run concurrently in the array. q- and k-side
            # iterations are interleaved so DVE (mask-add, normalize), ACT
            # (both exps), Pool (k-mask) and PE all have work in flight.
            with ExitStack() as ctxA, \
                 tc.tile_pool(name="spsum", bufs=4, space="PSUM") as sppool, \
                 tc.tile_pool(name="cpsum", bufs=2, space="PSUM") as cpool, \
                 tc.tile_pool(name="eq", bufs=6) as eqpool, \
                 tc.tile_pool(name="ee", bufs=4) as eepool, \
                 tc.tile_pool(name="ek", bufs=5) as ekpool, \
                 tc.tile_pool(name="rr", bufs=2) as rrpool, \
                 tc.tile_pool(name="bcp", bufs=2) as bcpool, \
                 tc.tile_pool(name="sm", bufs=8) as smpool:
                del ctxA

                def q_front(j, qt):
                    # PE scores -> DVE mask-add -> ACT exp(+sums)
                    c, hp = j // 2, (j % 2) * DH
                    esc = eqpool.tile([P, N], F32, tag="esc", name=f"esc{j}_{qt}")
                    for kh in range(2):
                        sq = sppool.tile([P, 512], F32, tag="sp",
                                         name=f"sq{j}_{qt}_{kh}")
                        nc.tensor.matmul(
                            sq[:],
                            lhsT=QT[hp:hp + DH, c, qt * P:(qt + 1) * P],
                            rhs=KT[hp:hp + DH, c, kh * 512:(kh + 1) * 512],
                            start=True, stop=True)
                        nc.vector.tensor_tensor(
                            esc[:, kh * 512:(kh + 1) * 512], sq[:],
                            MADD[:, qt, kh * 512:(kh + 1) * 512], ADD)
                    eexp = eepool.tile([P, N], F32, tag="eexp", name=f"ee{j}_{qt}")
                    sums = smpool.tile([P, 1], F32, tag="sums", name=f"sm{j}_{qt}")
                    nc.scalar.activation(eexp[:], esc[:], EXP, accum_out=sums[:])
                    return eexp, sums

                def q_back(j, qt, st):
                    # one step later: DVE recip+normalize, DMA out (no
                    # head-of-line blocking on the in-order DVE queue)
                    eexp, sums = st
                    rec = smpool.tile([P, 1], F32, tag="rec", name=f"rc{j}_{qt}")
                    nc.vector.reciprocal(rec[:], sums[:])
                    enorm = eqpool.tile([P, N], F32, tag="esc", name=f"en{j}_{qt}")
                    nc.vector.tensor_scalar_mul(enorm[:], eexp[:], rec[:])
                    nc.sync.dma_start(dist[qt * P:(qt + 1) * P, j, :], enorm[:])

                def k_front(j, kt):
                    # PE scores^T -> ACT exp -> Pool in-place mask
                    c, hp = j // 2, (j % 2) * DH
                    ekx = ekpool.tile([P, N], F32R, tag="ekx", name=f"ek{j}_{kt}")
                    for qh in range(2):
                        sk = sppool.tile([P, 512], F32, tag="sp",
                                         name=f"sk{j}_{kt}_{qh}")
                        nc.tensor.matmul(
                            sk[:],
                            lhsT=KT[hp:hp + DH, c, kt * P:(kt + 1) * P],
                            rhs=QT[hp:hp + DH, c, qh * 512:(qh + 1) * 512],
                            start=True, stop=True)
                        nc.scalar.activation(ekx[:, qh * 512:(qh + 1) * 512],
                                             sk[:], EXP)
                    # multiplicative mask in place (gpsimd reads/writes ekx);
                    # out keeps the f32r dtype so the AV matmul's producer
                    # check passes, input is read as plain f32 bits
                    nc.gpsimd.tensor_tensor(ekx[:], ekx.bitcast(F32)[:, :],
                                            MMUL[:, kt, :], MULT)
                    return ekx

                def k_av(j, kt, ctxp, ekx):
                    for qh in range(2):
                        nc.tensor.matmul(
                            ctxp[:, qh * 512:(qh + 1) * 512],
                            lhsT=V[:, kt, j, :],
                            rhs=ekx[:, qh * 512:(qh + 1) * 512],
                            start=(kt == 0), stop=(kt == KC - 1))

                def tail(j, ctxp):
                    c, hp = j // 2, (j % 2) * DH
                    rrow = rrpool.tile([1, N], F32, tag="rrow", name=f"rr{j}")
                    nc.vector.reciprocal(rrow[:], ctxp[DH:DH + 1, :])
                    # broadcast the reciprocal sums row across partitions
                    bc = bcpool.tile([DH, N], F32, tag="bc", name=f"bc{j}")
                    nc.gpsimd.partition_broadcast(bc[:], rrow[0:1, :])
                    nc.vector.tensor_tensor(CTX[hp:hp + DH, c, :],
                                            ctxp[0:DH, :], bc[:], MULT)

                # flat software pipeline across all pairs: fronts of step s
                # are emitted alongside the backs of step s-1, with no drain
                # at pair boundaries
                qs, ek, ctxs = {}, {}, {}
                steps = [(2 * p, 2 * p + 1, t)
                         for p in range(HG // 2) for t in range(8)]
                prev = None
                for j0, j1, t in steps:
                    if t == 0:
                        ctxs[j0] = cpool.tile([DH + 1, N], F32, tag="ctx",
                                              name=f"cx{j0}")
                        ctxs[j1] = cpool.tile([DH + 1, N], F32, tag="ctx",
                                              name=f"cx{j1}")
                    ek[(j0, t)] = k_front(j0, t)
                    qs[(j0, t)] = q_front(j0, t)
                    ek[(j1, t)] = k_front(j1, t)
                    qs[(j1, t)] = q_front(j1, t)
                    if t >= 1:
                        k_av(j0, t - 1, ctxs[j0], ek.pop((j0, t - 1)))
                        k_av(j1, t - 1, ctxs[j1], ek.pop((j1, t - 1)))
                        q_back(j0, t - 1, qs.pop((j0, t - 1)))
                        q_back(j1, t - 1, qs.pop((j1, t - 1)))
                    elif prev is not None:
                        p0, p1 = prev
                        k_av(p0, 7, ctxs[p0], ek.pop((p0, 7)))
                        k_av(p1, 7, ctxs[p1], ek.pop((p1, 7)))
                        q_back(p0, 7, qs.pop((p0, 7)))
                        q_back(p1, 7, qs.pop((p1, 7)))
                        tail(p0, ctxs.pop(p0))
                        tail(p1, ctxs.pop(p1))
                    prev = (j0, j1)
                p0, p1 = prev
                k_av(p0, 7, ctxs[p0], ek.pop((p0, 7)))
                k_av(p1, 7, ctxs[p1], ek.pop((p1, 7)))
                q_back(p0, 7, qs.pop((p0, 7)))
                q_back(p1, 7, qs.pop((p1, 7)))
                tail(p0, ctxs.pop(p0))
                tail(p1, ctxs.pop(p1))

        # ---------------- output projection ----------------
        with tc.tile_pool(name="opsum", bufs=4, space="PSUM") as oppool, \
             tc.tile_pool(name="ores", bufs=16) as orpool:
            for qt in range(8):
                for oh in range(2):
                    ps = oppool.tile([P, 512], F32, tag="op")
                    for hc in range(S // P):
                        nc.tensor.matmul(
                            ps[:],
                            lhsT=CTX[:, hc, qt * P:(qt + 1) * P],
                            rhs=WO[:, hc, oh * 512:(oh + 1) * 512],
                            start=(hc == 0), stop=(hc == S // P - 1))
                    rs = orpool.tile([P, 512], F32, tag="ores")
                    nc.scalar.copy(rs[:], ps[:])
                    nc.sync.dma_start(res[qt * P:(qt + 1) * P,
                                          oh * 512:(oh + 1) * 512], rs[:])

    nc.finalize()
    return nc


def prep_core_inputs(c, queries, keys, values, mask, Wq, bq, Wk, bk, Wv, bv,
                     Wo, bo):
    b, g = divmod(c, G)
    sl = slice(g * S, (g + 1) * S)
    r = _round_fp32r
    scale = 1.0 / np.sqrt(DH).astype(np.float32)
    return {
        "xqT": r(np.asarray(queries[b]).T),
        "xkT": r(np.asarray(keys[b]).T),
        "xvT": r(np.asarray(values[b]).T),
        "wqT": r((np.asarray(Wq[sl]) * scale).T),
        "wkT": r(np.asarray(Wk[sl]).T),
        "wvT": r(np.asarray(Wv[sl]).T),
        "bqr": np.ascontiguousarray(
            (np.asarray(bq[sl]) * scale).reshape(S // P, P).T.astype(np.float32)),
        "bkr": np.ascontiguousarray(
            np.asarray(bk[sl]).reshape(S // P, P).T.astype(np.float32)),
        "bvr": np.asarray(bv[sl]).reshape(1, S).astype(np.float32),
        "woT": r(np.asarray(Wo[:, sl]).T),
        "madd": np.where(np.asarray(mask[b]), np.float32(0.0),
                         np.float32(NEG)).astype(ml_dtypes.bfloat16),
        "mmulT": np.ascontiguousarray(
            np.asarray(mask[b]).T.astype(ml_dtypes.bfloat16)),
    }


def _ensure_axon_backend():
    # The SPMD runner executes via jax's axon backend; make sure a stray
    # JAX_PLATFORMS=cpu (used for running references) doesn't hide it.
    if os.environ.get("JAX_PLATFORMS", "").strip() == "cpu":
        os.environ.pop("JAX_PLATFORMS")
    try:
        import jax
        if not any("NC_" in str(d) for d in jax.devices()):
            jax.config.update("jax_platforms", "axon,cpu")
    except Exception:
        pass


def run_cores(in_maps, **kwargs):
    _ensure_axon_backend()
    if "nc" not in _CACHE:
        _CACHE["nc"] = build_program()
    return run_bass_kernel_spmd(_CACHE["nc"], in_maps, list(range(NCORES)),
                                **kwargs)


def kernel(queries, keys, values, mask, Wq, bq, Wk, bk, Wv, bv, Wo, bo):
    args = (queries, keys, values, mask, Wq, bq, Wk, bk, Wv, bv, Wo, bo)
    in_maps = [prep_core_inputs(c, *args) for c in range(NCORES)]
    outs = run_cores(in_maps).results

    results = np.empty((B, N, HID), np.float32)
    distribution = np.empty((B, N, H, N), np.float32)
    bo_np = np.asarray(bo, dtype=np.float32)
    for c in range(NCORES):
        b, g = divmod(c, G)
        distribution[b, :, g * HG:(g + 1) * HG, :] = outs[c]["dist"]
    for b in range(B):
        results[b] = outs[2 * b]["res"] + outs[2 * b + 1]["res"] + bo_np
    return results, distribution
